# revision 12
# baseline (speedup 1.0000x reference)
"""Trainium2 Bass kernel for the discrete-time Kuramoto oscillator classifier.

Computation (see harness reference):
    x: [4096, 64, 512] f32
    pooled = x.mean(axis=1)                      # [B, 512]
    theta0 = pooled @ W_in + b_in                # [B, 14]
    100 Kuramoto steps:
        dtheta = 2*pi*omega + (K/N)*(S*cos - C*sin),  S/C = per-sample sums
        theta += dt * dtheta
    h = cos(theta); logits = h @ Wc + bc; out = log_softmax(logits)

Sharding: pure data-parallel, batch split across 8 NeuronCores.

Device-side representation uses phase in *turns* (theta / 2pi), wrapped into
[-0.5, 0.5] each step with the fp32 magic-number round; ACT Sin evaluates
sin(2*pi*f) via its free input scale (Sin's valid domain is [-pi, pi]).
Oscillator groups live at 32-partition strides (engine/matmul partition-base
alignment): rows [32*g, 32*g+14) of a 128-partition tile hold the 14
oscillators of group g; a tile's free dim is a group-local batch column.
"""
import numpy as np

import concourse.bass as bass
import concourse.tile as tile
from concourse.tile_rust import add_dep_helper
from concourse import mybir
from concourse.bass_utils import run_bass_kernel_spmd
from waitsplit import split_multi_waits

FP32 = mybir.dt.float32
FP32R = mybir.dt.float32r
AF = mybir.ActivationFunctionType
ALU = mybir.AluOpType

# model constants (fixed by the problem)
N_OSC = 14
INPUT_DIM = 512
N_CLASSES = 2
COUPLING = 1.5
DT = 0.01
N_STEPS = 100
FREQS = np.array([2.0] * 2 + [6.0] * 4 + [40.0] * 8, dtype=np.float64)

B_TOTAL = 4096
SEQ = 64
N_CORES = 8

MAGIC = float(3 << 22)  # 12582912.0: x + M - M rounds x to nearest int (|x|<2^22)
TWO_PI = float(2.0 * np.pi)

# per-core tiling
B_CORE = B_TOTAL // N_CORES            # 512
TILE_B = 16                            # batches per x DMA tile (2 MiB)
N_XTILES = B_CORE // TILE_B            # 32
GROUP = 64                             # batches per oscillator group
N_GROUPS = B_CORE // GROUP             # 8
N_CHAINS = 2                           # recurrence chains (4 groups each)
GPC = N_GROUPS // N_CHAINS             # groups per chain = 4


def _build_consts(W_in, b_in, omega, Wc, bc):
    """Host-side constant tensors (np arrays) shared by all cores."""
    ones16 = np.zeros((128, 16), np.float32)
    for m in range(16):
        ones16[8 * m : 8 * m + 8, m] = 1.0 / SEQ

    wproj = np.zeros((128, 4 * N_OSC), np.float32)  # chunk c at cols [14c, 14c+14)
    for c in range(4):
        wproj[:, N_OSC * c : N_OSC * (c + 1)] = W_in[128 * c : 128 * (c + 1), :]

    ident = np.eye(128, dtype=np.float32)

    kcoup = np.zeros((128, 128), np.float32)
    kscale = DT * COUPLING / N_OSC / TWO_PI
    blk = np.full((N_OSC, N_OSC), kscale, np.float32)
    for g in range(4):
        kcoup[32 * g : 32 * g + N_OSC, 32 * g : 32 * g + N_OSC] = blk

    dtw = np.zeros((128, 1), np.float32)
    for g in range(4):
        dtw[32 * g : 32 * g + N_OSC, 0] = (DT * omega.astype(np.float64)) % 1.0

    initb = np.zeros((128, 2), np.float32)
    bturn = b_in.astype(np.float64) / TWO_PI
    for g in range(4):
        initb[32 * g : 32 * g + N_OSC, 0] = bturn
        initb[32 * g : 32 * g + N_OSC, 1] = bturn + 0.25

    wcls = np.zeros((128, 16), np.float32)
    for g in range(4):
        wcls[32 * g : 32 * g + N_OSC, 2 * g : 2 * g + N_CLASSES] = Wc.astype(
            np.float32
        )

    bcvec = np.zeros((16, 1), np.float32)
    for g in range(4):
        bcvec[2 * g : 2 * g + N_CLASSES, 0] = bc.astype(np.float32)

    pairones = np.zeros((16, 8), np.float32)
    pairbc = np.zeros((8, 16), np.float32)
    for g in range(8):
        pairones[2 * g : 2 * g + 2, g] = 1.0
        pairbc[g, 2 * g : 2 * g + 2] = 1.0

    return {
        "ones16": ones16,
        "wproj": wproj,
        "ident": ident,
        "kcoup": kcoup,
        "dtw": dtw,
        "initb": initb,
        "wcls": wcls,
        "bcvec": bcvec,
        "pairones": pairones,
        "pairbc": pairbc,
    }


def build_bass():
    nc = bass.Bass("TRN2", target_bir_lowering=False, debug=False)

    x_d = nc.declare_dram_parameter("x", [B_CORE, SEQ, INPUT_DIM], FP32R, isOutput=False)
    ones16_d = nc.declare_dram_parameter("ones16", [128, 16], FP32R, isOutput=False)
    wproj_d = nc.declare_dram_parameter("wproj", [128, 4 * N_OSC], FP32, isOutput=False)
    ident_d = nc.declare_dram_parameter("ident", [128, 128], FP32, isOutput=False)
    kcoup_d = nc.declare_dram_parameter("kcoup", [128, 128], FP32R, isOutput=False)
    dtw_d = nc.declare_dram_parameter("dtw", [128, 1], FP32, isOutput=False)
    initb_d = nc.declare_dram_parameter("initb", [128, 2], FP32, isOutput=False)
    wcls_d = nc.declare_dram_parameter("wcls", [128, 16], FP32, isOutput=False)
    bcvec_d = nc.declare_dram_parameter("bcvec", [16, 1], FP32, isOutput=False)
    pairones_d = nc.declare_dram_parameter("pairones", [16, 8], FP32, isOutput=False)
    pairbc_d = nc.declare_dram_parameter("pairbc", [8, 16], FP32, isOutput=False)
    out_d = nc.declare_dram_parameter("out", [16, 128], FP32, isOutput=True)



    with tile.TileContext(nc) as tc:
        with (
            tc.tile_pool(name="consts", bufs=1) as cpool,
            tc.tile_pool(name="xin", bufs=5) as xpool,
            tc.tile_pool(name="pool_sb", bufs=3) as ppool,
            tc.tile_pool(name="poolT_sb", bufs=3) as tpool,
            tc.tile_pool(name="tau", bufs=1) as taupool,
            tc.tile_pool(name="work", bufs=2) as wpool,
            tc.tile_pool(name="epi", bufs=1) as epool,
        ):
            # ---- constants ----
            ones16 = cpool.tile([128, 16], FP32R)
            nc.sync.dma_start(ones16[:], ones16_d[:])
            wproj = cpool.tile([128, 4 * N_OSC], FP32)  # chunk c at cols [14c,14c+14)
            nc.gpsimd.dma_start(wproj[:], wproj_d[:])
            ident = cpool.tile([128, 128], FP32)
            nc.gpsimd.dma_start(ident[:], ident_d[:])
            kcoup = cpool.tile([128, 128], FP32R)
            nc.sync.dma_start(kcoup[:], kcoup_d[:])
            dtw = cpool.tile([128, 1], FP32)
            nc.gpsimd.dma_start(dtw[:], dtw_d[:])
            magicv = cpool.tile([128, 1], FP32)
            nc.gpsimd.memset(magicv[:], MAGIC)
            initb = cpool.tile([128, 2], FP32)
            nc.gpsimd.dma_start(initb[:], initb_d[:])
            wcls = cpool.tile([128, 16], FP32)
            nc.gpsimd.dma_start(wcls[:], wcls_d[:])
            bcvec = cpool.tile([16, 1], FP32)
            nc.gpsimd.dma_start(bcvec[:], bcvec_d[:])
            pairones = cpool.tile([16, 8], FP32)
            nc.gpsimd.dma_start(pairones[:], pairones_d[:])
            pairbc = cpool.tile([8, 16], FP32)
            nc.gpsimd.dma_start(pairbc[:], pairbc_d[:])

            # ---- oscillator state: [tau | tau + 0.25] per chain, double buffered ----
            taus = []
            for h in range(N_CHAINS):
                pair = []
                for b in range(2):
                    t_ = taupool.tile([128, 2 * GROUP], FP32, tag=f"tau{h}{b}")
                    nc.gpsimd.memset(t_[:], 0.0)
                    pair.append(t_)
                taus.append(pair)

            # =================== phase A: pool + project ===================
            with (
                tc.tile_pool(name="pool_ps", bufs=3, space="PSUM") as pps,
                tc.tile_pool(name="theta_ps", bufs=2, space="PSUM") as thps,
                tc.tile_pool(name="tr_ps", bufs=2, space="PSUM") as trps,
            ):
                for g in range(N_GROUPS):
                    pooled_sb = ppool.tile([128, INPUT_DIM], FP32, tag="pooled")
                    for s in range(4):  # 4 x-tiles of 16 batches per group
                        t = 4 * g + s
                        xt = xpool.tile([128, 8 * INPUT_DIM], FP32R, tag="x")
                        nc.sync.dma_start(
                            xt[:],
                            x_d[16 * t : 16 * (t + 1), :, :].rearrange(
                                "b (ps q) d -> (b ps) (q d)", ps=8
                            ),
                        )
                        pool_ps = pps.tile([16, INPUT_DIM], FP32, tag="pp")
                        for j in range(8):
                            nc.tensor.matmul(
                                pool_ps[:, :],
                                ones16[:],
                                xt[:, j * INPUT_DIM : (j + 1) * INPUT_DIM],
                                start=(j == 0),
                                stop=(j == 7),
                            )
                        nc.scalar.activation(
                            pooled_sb[32 * s : 32 * s + 16, :], pool_ps[:, :], AF.Copy
                        )

                    # transpose pooled -> [d, batch-slot] chunks
                    pooledT = tpool.tile([128, INPUT_DIM], FP32, tag="pooledT")
                    for c in range(4):
                        tr_ps = trps.tile([128, 128], FP32, tag="tr")
                        nc.tensor.transpose(
                            tr_ps[:], pooled_sb[:, 128 * c : 128 * c + 128], ident[:]
                        )
                        nc.scalar.activation(
                            pooledT[:, 128 * c : 128 * c + 128], tr_ps[:], AF.Copy
                        )

                    # project: theta0^T for this group, [14, 64] (turns after copy)
                    th_ps = thps.tile([N_OSC, GROUP], FP32, tag="th")
                    pt4 = pooledT[:, :].rearrange("p (c q s) -> p c q s", c=4, q=4)
                    for c in range(4):
                        nc.tensor.matmul(
                            th_ps[:, :],
                            wproj[:, N_OSC * c : N_OSC * (c + 1)],
                            pt4[:, c, :, 0:16],
                            start=(c == 0),
                            stop=(c == 3),
                        )
                    # init tau halves (scale to turns, + b_in/2pi [+0.25])
                    h, gl = divmod(g, GPC)
                    tau = taus[h][0]
                    nc.scalar.activation(
                        tau[32 * gl : 32 * gl + N_OSC, 0:GROUP],
                        th_ps[:, :],
                        AF.Identity,
                        bias=initb[0:N_OSC, 0:1],
                        scale=1.0 / TWO_PI,
                    )
                    nc.scalar.activation(
                        tau[32 * gl : 32 * gl + N_OSC, GROUP : 2 * GROUP],
                        th_ps[:, :],
                        AF.Identity,
                        bias=initb[0:N_OSC, 1:2],
                        scale=1.0 / TWO_PI,
                    )

            # =================== phase B: recurrence ===================
            tc.strict_bb_all_engine_barrier()
            with tc.tile_pool(name="b_ps", bufs=2, space="PSUM") as bps:
                hc_tiles = []
                seq_last = {}

                def _seq(key, binst):
                    # pin per-engine issue order to emission order (no sems)
                    prev = seq_last.get(key)
                    if prev is not None:
                        add_dep_helper(binst.ins, prev.ins, sync=False,
                                       reason="chain-skew order")
                    seq_last[key] = binst
                    return binst

                def front(h, t):
                    # r2 -> f2 -> sin -> coupling MM
                    tau = taus[h][t % 2]
                    r2 = wpool.tile([128, 2 * GROUP], FP32, tag=f"r{h}", name=f"r_{h}_{t}")
                    _seq("act", nc.scalar.activation(r2[:], tau[:], AF.Identity, bias=magicv[:, :]))
                    f2 = wpool.tile([128, 2 * GROUP], FP32, tag=f"f{h}", name=f"f_{h}_{t}")
                    _seq("dve", nc.vector.scalar_tensor_tensor(
                        f2[:], r2[:], MAGIC, tau[:], op0=ALU.subtract, op1=ALU.subtract
                    ))
                    sc = wpool.tile([128, 2 * GROUP], FP32R, tag=f"s{h}", name=f"s_{h}_{t}")
                    _seq("act", nc.scalar.activation(sc[:], f2[:], AF.Sin, scale=-TWO_PI))
                    b_ps = bps.tile([128, 2 * GROUP], FP32, tag=f"b{h}", bufs=2, name=f"b_{h}_{t}")
                    _seq("pe", nc.tensor.matmul(b_ps[:, :], kcoup[:], sc[:], start=True, stop=True))
                    return sc, b_ps

                def back(h, t, sc, b_ps):
                    # m -> u -> tau update
                    tau = taus[h][t % 2]
                    tau_new = taus[h][(t + 1) % 2]
                    sc_swap = sc[:, :].rearrange("p (two f) -> p two f", two=2)[:, ::-1, :]
                    m = wpool.tile([128, 2 * GROUP], FP32, tag=f"m{h}", name=f"m_{h}_{t}")
                    _seq("dve", nc.vector.tensor_tensor(
                        m[:].rearrange("p (two f) -> p two f", two=2),
                        sc_swap,
                        b_ps[:, :].rearrange("p (two f) -> p two f", two=2),
                        op=ALU.mult,
                    ))
                    u = wpool.tile([128, GROUP], FP32, tag=f"d{h}", name=f"u_{h}_{t}")
                    _seq("dve", nc.vector.scalar_tensor_tensor(
                        u[:], m[:, 0:GROUP], dtw[:, :], m[:, GROUP:],
                        op0=ALU.add, op1=ALU.subtract,
                    ))
                    u_dup = u[:, :].unsqueeze(1).to_broadcast([128, 2, GROUP])
                    _seq("dve", nc.vector.tensor_add(
                        tau_new[:].rearrange("p (two f) -> p two f", two=2),
                        taus[h][t % 2][:].rearrange("p (two f) -> p two f", two=2),
                        u_dup,
                    ))

                # software-pipelined with half-step skew between chains
                pend = {0: None, 1: None}
                pend[0] = front(0, 0)
                for t in range(N_STEPS):
                    if pend[1] is not None:
                        back(1, t - 1, *pend[1])
                    pend[1] = front(1, t)
                    back(0, t, *pend[0])
                    if t + 1 < N_STEPS:
                        pend[0] = front(0, t + 1)
                back(1, N_STEPS - 1, *pend[1])

                # readout h = cos(theta_100) from the c-half of tau
                for h in range(N_CHAINS):
                    tau = taus[h][N_STEPS % 2]
                    rr = wpool.tile([128, GROUP], FP32, tag=f"rr{h}")
                    nc.scalar.activation(rr[:], tau[:, GROUP:], AF.Identity, bias=magicv[:, :])
                    ff = wpool.tile([128, GROUP], FP32, tag=f"ff{h}")
                    nc.vector.scalar_tensor_tensor(
                        ff[:], rr[:], MAGIC, tau[:, GROUP:], op0=ALU.subtract, op1=ALU.subtract
                    )
                    hc = wpool.tile([128, GROUP], FP32, tag=f"hc{h}")
                    nc.scalar.activation(hc[:], ff[:], AF.Sin, scale=-TWO_PI)
                    hc_tiles.append(hc)

                # classifier + exact 2-class log-softmax
                logit_ps = bps.tile([16, 2 * GROUP], FP32, tag="lg", bufs=1)
                for h in range(N_CHAINS):
                    nc.tensor.matmul(
                        logit_ps[:, GROUP * h : GROUP * (h + 1)],
                        wcls[:],
                        hc_tiles[h][:],
                        start=True,
                        stop=True,
                    )
                e_sb = epool.tile([16, 2 * GROUP], FP32, tag="e")
                nc.scalar.activation(e_sb[:], logit_ps[:, :], AF.Exp, bias=bcvec[:, :])
                l_sb = epool.tile([16, 2 * GROUP], FP32, tag="l")
                nc.scalar.activation(
                    l_sb[:], logit_ps[:, :], AF.Identity, bias=bcvec[:, :]
                )
                z_ps = bps.tile([8, 2 * GROUP], FP32, tag="z", bufs=1)
                nc.tensor.matmul(z_ps[:, :], pairones[:], e_sb[:], start=True, stop=True)
                lnz_sb = epool.tile([8, 2 * GROUP], FP32, tag="lnz")
                nc.scalar.activation(lnz_sb[:], z_ps[:, :], AF.Ln)
                zb_ps = bps.tile([16, 2 * GROUP], FP32, tag="zb", bufs=1)
                nc.tensor.matmul(
                    zb_ps[:, :], pairbc[:], lnz_sb[:], start=True, stop=True
                )
                out_sb = epool.tile([16, 2 * GROUP], FP32, tag="o")
                nc.vector.tensor_sub(out_sb[:], l_sb[:], zb_ps[:, :])
                nc.sync.dma_start(out_d[:], out_sb[:])

    split_multi_waits(nc)
    return nc


_CACHED = {}


def kernel(x, W_in, b_in, omega, Wc, bc):
    x = np.ascontiguousarray(x, dtype=np.float32)
    consts = _build_consts(
        np.asarray(W_in), np.asarray(b_in), np.asarray(omega), np.asarray(Wc),
        np.asarray(bc),
    )

    if "nc" not in _CACHED:
        _CACHED["nc"] = build_bass()
    nc = _CACHED["nc"]

    in_maps = []
    for c in range(N_CORES):
        m = {"x": x[c * B_CORE : (c + 1) * B_CORE]}
        m.update(consts)
        in_maps.append(m)

    res = run_bass_kernel_spmd(nc, in_maps, list(range(N_CORES)))
    _CACHED["last_results"] = res

    # out[2g'+j, 64h+n] -> batch 512c + 64*(4h+g') + n, class j
    out = np.empty((B_TOTAL, N_CLASSES), np.float32)
    for c in range(N_CORES):
        r = res.results[c]["out"][0:8]  # rows 2g'+j, cols 64h+n
        r4 = r.reshape(4, 2, 2, GROUP)  # [g', j, h, n]
        out[c * B_CORE : (c + 1) * B_CORE] = (
            r4.transpose(2, 0, 3, 1).reshape(B_CORE, N_CLASSES)
        )
    return out


# revision 13
# speedup vs baseline: 1.1251x; 1.1251x over previous
"""Trainium2 Bass kernel for the discrete-time Kuramoto oscillator classifier.

Computation (see harness reference):
    x: [4096, 64, 512] f32
    pooled = x.mean(axis=1)                      # [B, 512]
    theta0 = pooled @ W_in + b_in                # [B, 14]
    100 Kuramoto steps:
        dtheta = 2*pi*omega + (K/N)*(S*cos - C*sin),  S/C = per-sample sums
        theta += dt * dtheta
    h = cos(theta); logits = h @ Wc + bc; out = log_softmax(logits)

Sharding: pure data-parallel, batch split across 8 NeuronCores.

Device-side representation uses phase in *turns* (theta / 2pi), wrapped into
[-0.5, 0.5] each step with the fp32 magic-number round; ACT Sin evaluates
sin(2*pi*f) via its free input scale (Sin's valid domain is [-pi, pi]).
Oscillator groups live at 32-partition strides (engine/matmul partition-base
alignment): rows [32*g, 32*g+14) of a 128-partition tile hold the 14
oscillators of group g; a tile's free dim is a group-local batch column.
"""
import numpy as np

import concourse.bass as bass
import concourse.tile as tile
from concourse.tile_rust import add_dep_helper
from concourse import mybir
from concourse.bass_utils import run_bass_kernel_spmd
from waitsplit import split_multi_waits

FP32 = mybir.dt.float32
FP32R = mybir.dt.float32r
AF = mybir.ActivationFunctionType
ALU = mybir.AluOpType

# model constants (fixed by the problem)
N_OSC = 14
INPUT_DIM = 512
N_CLASSES = 2
COUPLING = 1.5
DT = 0.01
N_STEPS = 100
FREQS = np.array([2.0] * 2 + [6.0] * 4 + [40.0] * 8, dtype=np.float64)

B_TOTAL = 4096
SEQ = 64
N_CORES = 8

MAGIC = float(3 << 22)  # 12582912.0: x + M - M rounds x to nearest int (|x|<2^22)
TWO_PI = float(2.0 * np.pi)

# per-core tiling
B_CORE = B_TOTAL // N_CORES            # 512
TILE_B = 16                            # batches per x DMA tile (2 MiB)
N_XTILES = B_CORE // TILE_B            # 32
GROUP = 64                             # batches per oscillator group
N_GROUPS = B_CORE // GROUP             # 8
N_CHAINS = 2                           # recurrence chains (4 groups each)
GPC = N_GROUPS // N_CHAINS             # groups per chain = 4


def _build_consts(W_in, b_in, omega, Wc, bc):
    """Host-side constant tensors (np arrays) shared by all cores."""
    ones16 = np.zeros((128, 16), np.float32)
    for m in range(16):
        ones16[8 * m : 8 * m + 8, m] = 1.0 / SEQ

    wproj = np.zeros((128, 4 * N_OSC), np.float32)  # chunk c at cols [14c, 14c+14)
    for c in range(4):
        wproj[:, N_OSC * c : N_OSC * (c + 1)] = W_in[128 * c : 128 * (c + 1), :]

    ident = np.eye(128, dtype=np.float32)

    kcoup = np.zeros((128, 128), np.float32)
    kscale = DT * COUPLING / N_OSC / TWO_PI
    blk = np.full((N_OSC, N_OSC), kscale, np.float32)
    for g in range(4):
        kcoup[32 * g : 32 * g + N_OSC, 32 * g : 32 * g + N_OSC] = blk

    dtw = np.zeros((128, 1), np.float32)
    for g in range(4):
        dtw[32 * g : 32 * g + N_OSC, 0] = (DT * omega.astype(np.float64)) % 1.0

    initb = np.zeros((128, 2), np.float32)
    bturn = b_in.astype(np.float64) / TWO_PI
    for g in range(4):
        initb[32 * g : 32 * g + N_OSC, 0] = bturn
        initb[32 * g : 32 * g + N_OSC, 1] = bturn + 0.25

    wcls = np.zeros((128, 16), np.float32)
    for g in range(4):
        wcls[32 * g : 32 * g + N_OSC, 2 * g : 2 * g + N_CLASSES] = Wc.astype(
            np.float32
        )

    bcvec = np.zeros((16, 1), np.float32)
    for g in range(4):
        bcvec[2 * g : 2 * g + N_CLASSES, 0] = bc.astype(np.float32)

    pairones = np.zeros((16, 8), np.float32)
    pairbc = np.zeros((8, 16), np.float32)
    for g in range(8):
        pairones[2 * g : 2 * g + 2, g] = 1.0
        pairbc[g, 2 * g : 2 * g + 2] = 1.0

    return {
        "ones16": ones16,
        "wproj": wproj,
        "ident": ident,
        "kcoup": kcoup,
        "dtw": dtw,
        "initb": initb,
        "wcls": wcls,
        "bcvec": bcvec,
        "pairones": pairones,
        "pairbc": pairbc,
    }


def build_bass():
    nc = bass.Bass("TRN2", target_bir_lowering=False, debug=False)

    x_d = nc.declare_dram_parameter("x", [B_CORE, SEQ, INPUT_DIM], FP32R, isOutput=False)
    ones16_d = nc.declare_dram_parameter("ones16", [128, 16], FP32R, isOutput=False)
    wproj_d = nc.declare_dram_parameter("wproj", [128, 4 * N_OSC], FP32, isOutput=False)
    ident_d = nc.declare_dram_parameter("ident", [128, 128], FP32, isOutput=False)
    kcoup_d = nc.declare_dram_parameter("kcoup", [128, 128], FP32R, isOutput=False)
    dtw_d = nc.declare_dram_parameter("dtw", [128, 1], FP32, isOutput=False)
    initb_d = nc.declare_dram_parameter("initb", [128, 2], FP32, isOutput=False)
    wcls_d = nc.declare_dram_parameter("wcls", [128, 16], FP32, isOutput=False)
    bcvec_d = nc.declare_dram_parameter("bcvec", [16, 1], FP32, isOutput=False)
    pairones_d = nc.declare_dram_parameter("pairones", [16, 8], FP32, isOutput=False)
    pairbc_d = nc.declare_dram_parameter("pairbc", [8, 16], FP32, isOutput=False)
    out_d = nc.declare_dram_parameter("out", [16, 128], FP32, isOutput=True)



    with tile.TileContext(nc) as tc:
        with (
            tc.tile_pool(name="consts", bufs=1) as cpool,
            tc.tile_pool(name="xin", bufs=5) as xpool,
            tc.tile_pool(name="pool_sb", bufs=3) as ppool,
            tc.tile_pool(name="poolT_sb", bufs=3) as tpool,
            tc.tile_pool(name="tau", bufs=1) as taupool,
            tc.tile_pool(name="work", bufs=2) as wpool,
            tc.tile_pool(name="epi", bufs=1) as epool,
        ):
            # ---- constants ----
            ones16 = cpool.tile([128, 16], FP32R)
            nc.sync.dma_start(ones16[:], ones16_d[:])
            wproj = cpool.tile([128, 4 * N_OSC], FP32)  # chunk c at cols [14c,14c+14)
            nc.gpsimd.dma_start(wproj[:], wproj_d[:])
            ident = cpool.tile([128, 128], FP32)
            nc.gpsimd.dma_start(ident[:], ident_d[:])
            kcoup = cpool.tile([128, 128], FP32R)
            nc.sync.dma_start(kcoup[:], kcoup_d[:])
            dtw = cpool.tile([128, 1], FP32)
            nc.gpsimd.dma_start(dtw[:], dtw_d[:])
            magicv = cpool.tile([128, 1], FP32)
            nc.gpsimd.memset(magicv[:], MAGIC)
            initb = cpool.tile([128, 2], FP32)
            nc.gpsimd.dma_start(initb[:], initb_d[:])
            wcls = cpool.tile([128, 16], FP32)
            nc.gpsimd.dma_start(wcls[:], wcls_d[:])
            bcvec = cpool.tile([16, 1], FP32)
            nc.gpsimd.dma_start(bcvec[:], bcvec_d[:])
            pairones = cpool.tile([16, 8], FP32)
            nc.gpsimd.dma_start(pairones[:], pairones_d[:])
            pairbc = cpool.tile([8, 16], FP32)
            nc.gpsimd.dma_start(pairbc[:], pairbc_d[:])

            # ---- oscillator state: [tau | tau + 0.25] per chain, double buffered ----
            taus = []
            for h in range(N_CHAINS):
                pair = []
                for b in range(2):
                    t_ = taupool.tile([128, 2 * GROUP], FP32, tag=f"tau{h}{b}")
                    nc.gpsimd.memset(t_[:], 0.0)
                    pair.append(t_)
                taus.append(pair)

            # =================== phase A: pool + project ===================
            with (
                tc.tile_pool(name="pool_ps", bufs=3, space="PSUM") as pps,
                tc.tile_pool(name="theta_ps", bufs=2, space="PSUM") as thps,
                tc.tile_pool(name="tr_ps", bufs=2, space="PSUM") as trps,
            ):
                for g in range(N_GROUPS):
                    pooled_sb = ppool.tile([128, INPUT_DIM], FP32, tag="pooled")
                    for s in range(4):  # 4 x-tiles of 16 batches per group
                        t = 4 * g + s
                        xt = xpool.tile([128, 8 * INPUT_DIM], FP32R, tag="x")
                        nc.sync.dma_start(
                            xt[:],
                            x_d[16 * t : 16 * (t + 1), :, :].rearrange(
                                "b (ps q) d -> (b ps) (q d)", ps=8
                            ),
                        )
                        pool_ps = pps.tile([16, INPUT_DIM], FP32, tag="pp")
                        for j in range(8):
                            nc.tensor.matmul(
                                pool_ps[:, :],
                                ones16[:],
                                xt[:, j * INPUT_DIM : (j + 1) * INPUT_DIM],
                                start=(j == 0),
                                stop=(j == 7),
                            )
                        nc.scalar.activation(
                            pooled_sb[32 * s : 32 * s + 16, :], pool_ps[:, :], AF.Copy
                        )

                    # transpose pooled -> [d, batch-slot] chunks
                    pooledT = tpool.tile([128, INPUT_DIM], FP32, tag="pooledT")
                    for c in range(4):
                        tr_ps = trps.tile([128, 128], FP32, tag="tr")
                        nc.tensor.transpose(
                            tr_ps[:], pooled_sb[:, 128 * c : 128 * c + 128], ident[:]
                        )
                        nc.scalar.activation(
                            pooledT[:, 128 * c : 128 * c + 128], tr_ps[:], AF.Copy
                        )

                    # project: theta0^T for this group, [14, 64] (turns after copy)
                    th_ps = thps.tile([N_OSC, GROUP], FP32, tag="th")
                    pt4 = pooledT[:, :].rearrange("p (c q s) -> p c q s", c=4, q=4)
                    for c in range(4):
                        nc.tensor.matmul(
                            th_ps[:, :],
                            wproj[:, N_OSC * c : N_OSC * (c + 1)],
                            pt4[:, c, :, 0:16],
                            start=(c == 0),
                            stop=(c == 3),
                        )
                    # init tau halves (scale to turns, + b_in/2pi [+0.25])
                    h, gl = divmod(g, GPC)
                    tau = taus[h][0]
                    nc.scalar.activation(
                        tau[32 * gl : 32 * gl + N_OSC, 0:GROUP],
                        th_ps[:, :],
                        AF.Identity,
                        bias=initb[0:N_OSC, 0:1],
                        scale=1.0 / TWO_PI,
                    )
                    nc.scalar.activation(
                        tau[32 * gl : 32 * gl + N_OSC, GROUP : 2 * GROUP],
                        th_ps[:, :],
                        AF.Identity,
                        bias=initb[0:N_OSC, 1:2],
                        scale=1.0 / TWO_PI,
                    )

            # =================== phase B: recurrence ===================
            tc.strict_bb_all_engine_barrier()
            with tc.tile_pool(name="b_ps", bufs=2, space="PSUM") as bps:
                hc_tiles = []
                seq_last = {}

                def _seq(key, binst):
                    return binst

                def front(h, t):
                    # r2 -> f2 -> sin -> coupling MM
                    tau = taus[h][t % 2]
                    r2 = wpool.tile([128, 2 * GROUP], FP32, tag=f"r{h}", name=f"r_{h}_{t}")
                    _seq("dve", nc.vector.tensor_scalar(
                        r2[:], tau[:], MAGIC, MAGIC, op0=ALU.add, op1=ALU.subtract
                    ))
                    f2 = wpool.tile([128, 2 * GROUP], FP32, tag=f"f{h}", name=f"f_{h}_{t}")
                    _seq("dve", nc.vector.tensor_sub(f2[:], tau[:], r2[:]))
                    sc = wpool.tile([128, 2 * GROUP], FP32R, tag=f"s{h}", name=f"s_{h}_{t}")
                    _seq("act", nc.scalar.activation(sc[:], f2[:], AF.Sin, scale=TWO_PI))
                    b_ps = bps.tile([128, 2 * GROUP], FP32, tag=f"b{h}", bufs=2, name=f"b_{h}_{t}")
                    _seq("pe", nc.tensor.matmul(b_ps[:, :], kcoup[:], sc[:], start=True, stop=True))
                    return sc, b_ps

                def back(h, t, sc, b_ps):
                    # m -> u -> tau update
                    tau = taus[h][t % 2]
                    tau_new = taus[h][(t + 1) % 2]
                    sc_swap = sc[:, :].rearrange("p (two f) -> p two f", two=2)[:, ::-1, :]
                    m = wpool.tile([128, 2 * GROUP], FP32, tag=f"m{h}", name=f"m_{h}_{t}")
                    _seq("dve", nc.vector.tensor_tensor(
                        m[:].rearrange("p (two f) -> p two f", two=2),
                        sc_swap,
                        b_ps[:, :].rearrange("p (two f) -> p two f", two=2),
                        op=ALU.mult,
                    ))
                    u = wpool.tile([128, GROUP], FP32, tag=f"d{h}", name=f"u_{h}_{t}")
                    _seq("dve", nc.vector.scalar_tensor_tensor(
                        u[:], m[:, 0:GROUP], dtw[:, :], m[:, GROUP:],
                        op0=ALU.add, op1=ALU.subtract,
                    ))
                    u_dup = u[:, :].unsqueeze(1).to_broadcast([128, 2, GROUP])
                    _seq("dve", nc.vector.tensor_add(
                        tau_new[:].rearrange("p (two f) -> p two f", two=2),
                        taus[h][t % 2][:].rearrange("p (two f) -> p two f", two=2),
                        u_dup,
                    ))

                # software-pipelined with half-step skew between chains
                pend = {0: None, 1: None}
                pend[0] = front(0, 0)
                for t in range(N_STEPS):
                    if pend[1] is not None:
                        back(1, t - 1, *pend[1])
                    pend[1] = front(1, t)
                    back(0, t, *pend[0])
                    if t + 1 < N_STEPS:
                        pend[0] = front(0, t + 1)
                back(1, N_STEPS - 1, *pend[1])

                # readout h = cos(theta_100) from the c-half of tau
                for h in range(N_CHAINS):
                    tau = taus[h][N_STEPS % 2]
                    rr = wpool.tile([128, GROUP], FP32, tag=f"rr{h}")
                    nc.scalar.activation(rr[:], tau[:, GROUP:], AF.Identity, bias=magicv[:, :])
                    ff = wpool.tile([128, GROUP], FP32, tag=f"ff{h}")
                    nc.vector.scalar_tensor_tensor(
                        ff[:], rr[:], MAGIC, tau[:, GROUP:], op0=ALU.subtract, op1=ALU.subtract
                    )
                    hc = wpool.tile([128, GROUP], FP32, tag=f"hc{h}")
                    nc.scalar.activation(hc[:], ff[:], AF.Sin, scale=-TWO_PI)
                    hc_tiles.append(hc)

                # classifier + exact 2-class log-softmax
                logit_ps = bps.tile([16, 2 * GROUP], FP32, tag="lg", bufs=1)
                for h in range(N_CHAINS):
                    nc.tensor.matmul(
                        logit_ps[:, GROUP * h : GROUP * (h + 1)],
                        wcls[:],
                        hc_tiles[h][:],
                        start=True,
                        stop=True,
                    )
                e_sb = epool.tile([16, 2 * GROUP], FP32, tag="e")
                nc.scalar.activation(e_sb[:], logit_ps[:, :], AF.Exp, bias=bcvec[:, :])
                l_sb = epool.tile([16, 2 * GROUP], FP32, tag="l")
                nc.scalar.activation(
                    l_sb[:], logit_ps[:, :], AF.Identity, bias=bcvec[:, :]
                )
                z_ps = bps.tile([8, 2 * GROUP], FP32, tag="z", bufs=1)
                nc.tensor.matmul(z_ps[:, :], pairones[:], e_sb[:], start=True, stop=True)
                lnz_sb = epool.tile([8, 2 * GROUP], FP32, tag="lnz")
                nc.scalar.activation(lnz_sb[:], z_ps[:, :], AF.Ln)
                zb_ps = bps.tile([16, 2 * GROUP], FP32, tag="zb", bufs=1)
                nc.tensor.matmul(
                    zb_ps[:, :], pairbc[:], lnz_sb[:], start=True, stop=True
                )
                out_sb = epool.tile([16, 2 * GROUP], FP32, tag="o")
                nc.vector.tensor_sub(out_sb[:], l_sb[:], zb_ps[:, :])
                nc.sync.dma_start(out_d[:], out_sb[:])

    split_multi_waits(nc)
    return nc


_CACHED = {}


def kernel(x, W_in, b_in, omega, Wc, bc):
    x = np.ascontiguousarray(x, dtype=np.float32)
    consts = _build_consts(
        np.asarray(W_in), np.asarray(b_in), np.asarray(omega), np.asarray(Wc),
        np.asarray(bc),
    )

    if "nc" not in _CACHED:
        _CACHED["nc"] = build_bass()
    nc = _CACHED["nc"]

    in_maps = []
    for c in range(N_CORES):
        m = {"x": x[c * B_CORE : (c + 1) * B_CORE]}
        m.update(consts)
        in_maps.append(m)

    res = run_bass_kernel_spmd(nc, in_maps, list(range(N_CORES)))
    _CACHED["last_results"] = res

    # out[2g'+j, 64h+n] -> batch 512c + 64*(4h+g') + n, class j
    out = np.empty((B_TOTAL, N_CLASSES), np.float32)
    for c in range(N_CORES):
        r = res.results[c]["out"][0:8]  # rows 2g'+j, cols 64h+n
        r4 = r.reshape(4, 2, 2, GROUP)  # [g', j, h, n]
        out[c * B_CORE : (c + 1) * B_CORE] = (
            r4.transpose(2, 0, 3, 1).reshape(B_CORE, N_CLASSES)
        )
    return out
